# revision 1
# baseline (speedup 1.0000x reference)
"""ColBERT late-interaction scoring kernel for Trainium2 (Bass/Tile).

score_b = sum_q max_k (2*Q@D^T - ||q||^2 - ||d||^2)
        = sum_q max_k (2*qd[q,k] - d_sq[k])  -  ||Q_b||_F^2

Sharding: data-parallel over batch B=128 across 8 NeuronCores (16 each).

Per-core per-batch device pipeline:
  1. SWDGE cast-DMA loads Q,D (f32 DRAM -> bf16 SBUF, natural layout)
  2. HWDGE xbar-transpose DMAs build QT,DT [d=128, L=512] bf16
  3. GPSIMD computes SQ = DT*DT
  4. PE matmul (lhsT = -0.5*ones) broadcasts -0.5*d_sq into a PSUM bank
  5. PE matmuls (lhsT = QT tile) produce qd [128q, 512k] PSUM tiles
  6. DVE tensor_tensor_reduce fuses (qd - 0.5*d_sq)*2 + max_k per tile
  7. DVE ttr accumulates per-partition ||Q||^2 partials
  Endgame: small reduce + ones-matmul partition sum -> [16] scores.
"""

import numpy as np

B, LQ, LD, D = 128, 512, 512, 128
N_CORES = 8
BPC = B // N_CORES  # batches per core
NT = LQ // 128  # q tiles per batch

_compiled = {}


def _cast_gpsimd(nc, dst, src):
    nc.gpsimd.tensor_copy(dst, src)


def _cast_act(nc, dst, src):
    nc.scalar.copy(dst, src)


def _cast_dve(nc, dst, src):
    nc.vector.tensor_copy(dst, src)


_CAST_Q = _cast_gpsimd
_CAST_D = _cast_gpsimd


def _split_multi_waits(nc):
    """This container's walrus accepts only ONE sem-wait per instruction
    (setupSyncWait: 'Too many sync wait commands'). Tile's wait assignment
    emits multi-wait instructions, so split: every extra wait moves onto a
    dedicated NoOp inserted just before the instruction on the same engine.
    Engine program order makes this semantically identical."""
    import concourse.mybir as mybir

    for f in nc.m.functions:
        for blk in f.blocks:
            il = blk.instructions
            i = 0
            while i < len(il):
                inst = il[i]
                si = inst.sync_info
                waits = list(si.on_wait) if si and si.on_wait else []
                if len(waits) > 1:
                    for w in waits[:-1]:
                        nop = mybir.InstNoOp(
                            name=nc.get_next_instruction_name(), ins=[], outs=[]
                        )
                        nop.engine = inst.engine
                        nop.sync_info = mybir.SyncInfo(on_wait=[w], on_update=[])
                        il.insert(i, nop)
                        i += 1
                    inst.sync_info = mybir.SyncInfo(
                        on_wait=[waits[-1]], on_update=si.on_update
                    )
                i += 1


def _build(reps: int = 1):
    import concourse.bass as bass
    import concourse.mybir as mybir
    import concourse.tile as tile
    from concourse.bass import ts

    nc = bass.Bass()
    f32 = mybir.dt.float32
    bf16 = mybir.dt.bfloat16

    qe = nc.dram_tensor("qe", [BPC, LQ, D], f32, kind="ExternalInput")
    de = nc.dram_tensor("de", [BPC, LD, D], f32, kind="ExternalInput")
    out = nc.dram_tensor("out", [1, BPC], f32, kind="ExternalOutput")

    with tile.TileContext(nc) as tc:
        with (
            tc.tile_pool(name="consts", bufs=1) as cpool,
            tc.tile_pool(name="work", bufs=3) as wpool,
            tc.tile_pool(name="acc", bufs=1) as apool,
            tc.tile_pool(name="ps", bufs=4, space="PSUM") as pspool,
        ):
            neg_half = cpool.tile([128, 128], bf16)
            nc.gpsimd.memset(neg_half, -0.5)
            ones_col = cpool.tile([128, 1], f32)
            nc.gpsimd.memset(ones_col, 1.0)

            # rowmax accumulator [128, BPC*NT] and qsq accumulator [128, BPC]
            MX = apool.tile([128, BPC * NT], f32)
            QS = apool.tile([128, BPC], f32)

            GRP = 4  # batches per load/cast group
            NG = BPC // GRP
            for rep in range(reps):
                # Phase 1: HWDGE f32 loads, 4 batches per DMA (SWDGE
                # cast-DMAs measured ~2ms each here — pathological). Layout
                # q = NT*p + t: partition p covers NT consecutive DRAM rows =
                # one contiguous run. Tile t then holds q's {NT*p + t}, a
                # harmless reordering (scores sum over all q).
                qbs, dbs = [], []
                for g in range(NG):
                    qf = wpool.tile([128, GRP, NT, 128], f32, tag="qf")
                    df = wpool.tile([128, GRP, NT, 128], f32, tag="df")
                    nc.sync.dma_start(
                        qf, qe[g * GRP : (g + 1) * GRP].rearrange("b (p t) d -> p b t d", t=NT)
                    )
                    nc.sync.dma_start(
                        df, de[g * GRP : (g + 1) * GRP].rearrange("b (p t) d -> p b t d", t=NT)
                    )
                    # Phase 2: cast f32 -> bf16 (gpsimd; 1-input ~line rate).
                    # bf16 tiles get a fresh slot per group: WAR waits would
                    # land on XPOSE DMAs, which allow a single sem-wait.
                    qb = wpool.tile([128, GRP, NT, 128], bf16, tag="qb", bufs=NG)
                    db = wpool.tile([128, GRP, NT, 128], bf16, tag="db", bufs=NG)
                    _CAST_Q(nc, qb, qf)
                    _CAST_D(nc, db, df)
                    qbs.append(qb)
                    dbs.append(db)

                # Phase 3: xbar transposes -> QT/DT [d, L] per batch
                QTs, DTs = [], []
                for b in range(BPC):
                    g, i = b // GRP, b % GRP
                    QT = wpool.tile([128, LQ], bf16, tag="QT", bufs=BPC)
                    DT = wpool.tile([128, LD], bf16, tag="DT", bufs=BPC)
                    for t in range(NT):
                        nc.sync.dma_start_transpose(QT[:, ts(t, 128)], qbs[g][:, i, t, :])
                        nc.sync.dma_start_transpose(DT[:, ts(t, 128)], dbs[g][:, i, t, :])
                    QTs.append(QT)
                    DTs.append(DT)

                # Phase 4: SQ = DT*DT and qsq accumulation (ScalarE)
                SQs = []
                for b in range(BPC):
                    SQ = wpool.tile([128, LD], bf16, tag="SQ", bufs=BPC)
                    nc.scalar.activation(SQ, DTs[b], mybir.ActivationFunctionType.Square)
                    SQs.append(SQ)
                    g, i = b // GRP, b % GRP
                    junkb = wpool.tile([128, NT, 128], bf16, tag="junkb")
                    nc.scalar.activation(
                        junkb,
                        qbs[g][:, i],
                        mybir.ActivationFunctionType.Square,
                        accum_out=QS[:, b : b + 1],
                    )

                # Phase 5: per batch, 4 accumulation groups (bias bcast + qd)
                # split across two 2-bank psum tiles (bufs=4), each closed by
                # a fused rowmax reduce. Finer PSUM granularity lets PE start
                # the next half-batch while DVE reduces the previous one —
                # measured ~19% faster than one 4-bank tile x 2 bufs.
                # (Measured alternatives on this runtime: single matmuls +
                # DVE bias-add in PSUM serialize PE<->DVE at tile granularity
                # and are ~2x slower overall than the accumulation groups.)
                for b in range(BPC):
                  for h in range(2):
                    pst = pspool.tile([128, NT // 2, LD], f32, tag="pst")
                    for t2 in range(NT // 2):
                        t = h * (NT // 2) + t2
                        nc.tensor.matmul(
                            pst[:, t2, :], lhsT=neg_half, rhs=SQs[b],
                            start=True, stop=False,
                        )
                        nc.tensor.matmul(
                            pst[:, t2, :],
                            lhsT=QTs[b][:, ts(t, 128)],
                            rhs=DTs[b],
                            start=False,
                            stop=True,
                        )
                    nc.vector.reduce_max(
                        MX[:, b * NT + h * (NT // 2) : b * NT + (h + 1) * (NT // 2)],
                        pst, axis=mybir.AxisListType.X
                    )

            # Endgame: SC[p, b] = 2 * sum_t MX[p, b*NT+t] - QS[p, b]
            msum = apool.tile([128, BPC], f32)
            nc.vector.reduce_sum(
                msum, MX.rearrange("p (b t) -> p b t", t=NT), axis=mybir.AxisListType.X
            )
            msum2 = apool.tile([128, BPC], f32)
            nc.vector.tensor_scalar_mul(msum2, msum, 2.0)
            sc = apool.tile([128, BPC], f32)
            nc.vector.tensor_tensor(sc, msum2, QS, op=mybir.AluOpType.subtract)

            # partition sum via ones-matmul -> [1, BPC] (reuses a pst slot)
            ps_s = pspool.tile([1, BPC], f32, tag="pst")
            nc.tensor.matmul(ps_s, lhsT=ones_col, rhs=sc, start=True, stop=True)
            score = apool.tile([1, BPC], f32)
            nc.vector.tensor_copy(score, ps_s)
            nc.sync.dma_start(out[:, :], score)

    _split_multi_waits(nc)
    return nc


def kernel(query_embedding: np.ndarray, document_embedding: np.ndarray) -> np.ndarray:
    from concourse.bass_utils import run_bass_kernel_spmd

    if "nc" not in _compiled:
        _compiled["nc"] = _build()
    nc = _compiled["nc"]

    qe = np.ascontiguousarray(query_embedding, dtype=np.float32)
    de = np.ascontiguousarray(document_embedding, dtype=np.float32)
    in_maps = [
        {"qe": qe[c * BPC : (c + 1) * BPC], "de": de[c * BPC : (c + 1) * BPC]}
        for c in range(N_CORES)
    ]
    res = run_bass_kernel_spmd(nc, in_maps, core_ids=list(range(N_CORES)))
    return np.concatenate(
        [res.results[c]["out"].reshape(BPC) for c in range(N_CORES)]
    ).astype(np.float32)



# revision 2
# speedup vs baseline: 9.6079x; 9.6079x over previous
"""ColBERT late-interaction scoring kernel for Trainium2 (Bass/Tile).

score_b = sum_q max_k (2*Q@D^T - ||q||^2 - ||d||^2)
        = sum_q max_k (2*qd[q,k] - d_sq[k])  -  ||Q_b||_F^2

Sharding: data-parallel over batch B=128 across 8 NeuronCores (16 each).

Per-core per-batch device pipeline:
  1. SWDGE cast-DMA loads Q,D (f32 DRAM -> bf16 SBUF, natural layout)
  2. HWDGE xbar-transpose DMAs build QT,DT [d=128, L=512] bf16
  3. GPSIMD computes SQ = DT*DT
  4. PE matmul (lhsT = -0.5*ones) broadcasts -0.5*d_sq into a PSUM bank
  5. PE matmuls (lhsT = QT tile) produce qd [128q, 512k] PSUM tiles
  6. DVE tensor_tensor_reduce fuses (qd - 0.5*d_sq)*2 + max_k per tile
  7. DVE ttr accumulates per-partition ||Q||^2 partials
  Endgame: small reduce + ones-matmul partition sum -> [16] scores.
"""

import numpy as np

B, LQ, LD, D = 128, 512, 512, 128
N_CORES = 8
BPC = B // N_CORES  # batches per core
NT = LQ // 128  # q tiles per batch

_compiled = {}


def _cast_gpsimd(nc, dst, src):
    nc.gpsimd.tensor_copy(dst, src)


def _cast_act(nc, dst, src):
    nc.scalar.copy(dst, src)


def _cast_dve(nc, dst, src):
    nc.vector.tensor_copy(dst, src)


_CAST_Q = _cast_gpsimd
_CAST_D = _cast_gpsimd


def _split_multi_waits(nc):
    """This container's walrus accepts only ONE sem-wait per instruction
    (setupSyncWait: 'Too many sync wait commands'). Tile's wait assignment
    emits multi-wait instructions, so split: every extra wait moves onto a
    dedicated NoOp inserted just before the instruction on the same engine.
    Engine program order makes this semantically identical."""
    import concourse.mybir as mybir

    for f in nc.m.functions:
        for blk in f.blocks:
            il = blk.instructions
            i = 0
            while i < len(il):
                inst = il[i]
                si = inst.sync_info
                waits = list(si.on_wait) if si and si.on_wait else []
                if len(waits) > 1:
                    for w in waits[:-1]:
                        nop = mybir.InstNoOp(
                            name=nc.get_next_instruction_name(), ins=[], outs=[]
                        )
                        nop.engine = inst.engine
                        nop.sync_info = mybir.SyncInfo(on_wait=[w], on_update=[])
                        il.insert(i, nop)
                        i += 1
                    inst.sync_info = mybir.SyncInfo(
                        on_wait=[waits[-1]], on_update=si.on_update
                    )
                i += 1


def _build(reps: int = 1):
    import contextlib

    import concourse.bass as bass
    import concourse.mybir as mybir
    import concourse.tile as tile
    from concourse.bass import ts

    nc = bass.Bass()
    f32 = mybir.dt.float32
    bf16 = mybir.dt.bfloat16

    qe = nc.dram_tensor("qe", [BPC, LQ, D], f32, kind="ExternalInput")
    de = nc.dram_tensor("de", [BPC, LD, D], f32, kind="ExternalInput")
    out = nc.dram_tensor("out", [1, BPC], f32, kind="ExternalOutput")

    with tile.TileContext(nc) as tc:
        with (
            tc.tile_pool(name="consts", bufs=1) as cpool,
            tc.tile_pool(name="work", bufs=3) as wpool,
            tc.tile_pool(name="acc", bufs=1) as apool,
            tc.tile_pool(name="ps", bufs=4, space="PSUM") as pspool,
        ):
            neg_half = cpool.tile([128, 128], bf16)
            nc.gpsimd.memset(neg_half, -0.5)
            ones_col = cpool.tile([128, 1], f32)
            nc.gpsimd.memset(ones_col, 1.0)

            # rowmax accumulator [128, BPC*NT] and qsq accumulator [128, BPC]
            MX = apool.tile([128, BPC * NT], f32)
            QS = apool.tile([128, BPC], f32)

            GRP = 4  # batches per load/cast group
            NG = BPC // GRP
            # reps>1 is the timing mode: run the identical body in a HARDWARE
            # loop so program size (and thus NEFF load time) stays constant
            # across reps — the wall-clock differential then isolates actual
            # steady-state device execution instead of NEFF-size overhead.
            rep_ctx = tc.For_i(0, reps) if reps > 1 else contextlib.nullcontext()
            with rep_ctx:
                # Phase 1: HWDGE f32 loads, 4 batches per DMA (SWDGE
                # cast-DMAs measured ~2ms each here — pathological). Layout
                # q = NT*p + t: partition p covers NT consecutive DRAM rows =
                # one contiguous run. Tile t then holds q's {NT*p + t}, a
                # harmless reordering (scores sum over all q).
                qbs, dbs = [], []
                for g in range(NG):
                    qf = wpool.tile([128, GRP, NT, 128], f32, tag="qf")
                    df = wpool.tile([128, GRP, NT, 128], f32, tag="df")
                    nc.sync.dma_start(
                        qf, qe[g * GRP : (g + 1) * GRP].rearrange("b (p t) d -> p b t d", t=NT)
                    )
                    nc.sync.dma_start(
                        df, de[g * GRP : (g + 1) * GRP].rearrange("b (p t) d -> p b t d", t=NT)
                    )
                    # Phase 2: cast f32 -> bf16 (gpsimd; 1-input ~line rate).
                    # bf16 tiles get a fresh slot per group: WAR waits would
                    # land on XPOSE DMAs, which allow a single sem-wait.
                    qb = wpool.tile([128, GRP, NT, 128], bf16, tag="qb", bufs=NG)
                    db = wpool.tile([128, GRP, NT, 128], bf16, tag="db", bufs=NG)
                    _CAST_Q(nc, qb, qf)
                    _CAST_D(nc, db, df)
                    qbs.append(qb)
                    dbs.append(db)

                # Phase 3: xbar transposes -> QT/DT [d, L] per batch
                QTs, DTs = [], []
                for b in range(BPC):
                    g, i = b // GRP, b % GRP
                    QT = wpool.tile([128, LQ], bf16, tag="QT", bufs=BPC)
                    DT = wpool.tile([128, LD], bf16, tag="DT", bufs=BPC)
                    for t in range(NT):
                        nc.sync.dma_start_transpose(QT[:, ts(t, 128)], qbs[g][:, i, t, :])
                        nc.sync.dma_start_transpose(DT[:, ts(t, 128)], dbs[g][:, i, t, :])
                    QTs.append(QT)
                    DTs.append(DT)

                # Phase 4: SQ = DT*DT and qsq accumulation (ScalarE)
                SQs = []
                for b in range(BPC):
                    SQ = wpool.tile([128, LD], bf16, tag="SQ", bufs=BPC)
                    nc.scalar.activation(SQ, DTs[b], mybir.ActivationFunctionType.Square)
                    SQs.append(SQ)
                    g, i = b // GRP, b % GRP
                    junkb = wpool.tile([128, NT, 128], bf16, tag="junkb")
                    nc.scalar.activation(
                        junkb,
                        qbs[g][:, i],
                        mybir.ActivationFunctionType.Square,
                        accum_out=QS[:, b : b + 1],
                    )

                # Phase 5: per batch, 4 accumulation groups (bias bcast + qd)
                # split across two 2-bank psum tiles (bufs=4), each closed by
                # a fused rowmax reduce. Finer PSUM granularity lets PE start
                # the next half-batch while DVE reduces the previous one —
                # measured ~19% faster than one 4-bank tile x 2 bufs.
                # (Measured alternatives on this runtime: single matmuls +
                # DVE bias-add in PSUM serialize PE<->DVE at tile granularity
                # and are ~2x slower overall than the accumulation groups.)
                for b in range(BPC):
                  for h in range(2):
                    pst = pspool.tile([128, NT // 2, LD], f32, tag="pst")
                    for t2 in range(NT // 2):
                        t = h * (NT // 2) + t2
                        nc.tensor.matmul(
                            pst[:, t2, :], lhsT=neg_half, rhs=SQs[b],
                            start=True, stop=False,
                        )
                        nc.tensor.matmul(
                            pst[:, t2, :],
                            lhsT=QTs[b][:, ts(t, 128)],
                            rhs=DTs[b],
                            start=False,
                            stop=True,
                        )
                    nc.vector.reduce_max(
                        MX[:, b * NT + h * (NT // 2) : b * NT + (h + 1) * (NT // 2)],
                        pst, axis=mybir.AxisListType.X
                    )

            # Endgame: SC[p, b] = 2 * sum_t MX[p, b*NT+t] - QS[p, b]
            msum = apool.tile([128, BPC], f32)
            nc.vector.reduce_sum(
                msum, MX.rearrange("p (b t) -> p b t", t=NT), axis=mybir.AxisListType.X
            )
            msum2 = apool.tile([128, BPC], f32)
            nc.vector.tensor_scalar_mul(msum2, msum, 2.0)
            sc = apool.tile([128, BPC], f32)
            nc.vector.tensor_tensor(sc, msum2, QS, op=mybir.AluOpType.subtract)

            # partition sum via ones-matmul -> [1, BPC] (reuses a pst slot)
            ps_s = pspool.tile([1, BPC], f32, tag="pst")
            nc.tensor.matmul(ps_s, lhsT=ones_col, rhs=sc, start=True, stop=True)
            score = apool.tile([1, BPC], f32)
            nc.vector.tensor_copy(score, ps_s)
            nc.sync.dma_start(out[:, :], score)

    _split_multi_waits(nc)
    return nc


def kernel(query_embedding: np.ndarray, document_embedding: np.ndarray) -> np.ndarray:
    from concourse.bass_utils import run_bass_kernel_spmd

    if "nc" not in _compiled:
        _compiled["nc"] = _build()
    nc = _compiled["nc"]

    qe = np.ascontiguousarray(query_embedding, dtype=np.float32)
    de = np.ascontiguousarray(document_embedding, dtype=np.float32)
    in_maps = [
        {"qe": qe[c * BPC : (c + 1) * BPC], "de": de[c * BPC : (c + 1) * BPC]}
        for c in range(N_CORES)
    ]
    res = run_bass_kernel_spmd(nc, in_maps, core_ids=list(range(N_CORES)))
    return np.concatenate(
        [res.results[c]["out"].reshape(BPC) for c in range(N_CORES)]
    ).astype(np.float32)



# revision 17
# speedup vs baseline: 342.9378x; 35.6934x over previous
"""ColBERT late-interaction scoring kernel for Trainium2 (Bass/Tile).

score_b = sum_q max_k (2*Q@D^T - ||q||^2 - ||d||^2)
        = sum_q max_k (2*qd[q,k] - d_sq[k])  -  ||Q_b||_F^2

Sharding: data-parallel over batch B=128 across 8 NeuronCores (16 each).

Per-core per-batch device pipeline:
  1. SWDGE cast-DMA loads Q,D (f32 DRAM -> bf16 SBUF, natural layout)
  2. HWDGE xbar-transpose DMAs build QT,DT [d=128, L=512] bf16
  3. GPSIMD computes SQ = DT*DT
  4. PE matmul (lhsT = -0.5*ones) broadcasts -0.5*d_sq into a PSUM bank
  5. PE matmuls (lhsT = QT tile) produce qd [128q, 512k] PSUM tiles
  6. DVE tensor_tensor_reduce fuses (qd - 0.5*d_sq)*2 + max_k per tile
  7. DVE ttr accumulates per-partition ||Q||^2 partials
  Endgame: small reduce + ones-matmul partition sum -> [16] scores.
"""

import numpy as np

B, LQ, LD, D = 128, 512, 512, 128
N_CORES = 8
BPC = B // N_CORES  # batches per core
NT = LQ // 128  # q tiles per batch

_compiled = {}


def _cast_gpsimd(nc, dst, src):
    nc.gpsimd.tensor_copy(dst, src)


def _cast_act(nc, dst, src):
    nc.scalar.copy(dst, src)


def _cast_dve(nc, dst, src):
    nc.vector.tensor_copy(dst, src)


_CAST_Q = _cast_gpsimd
_CAST_D = _cast_gpsimd


def _split_multi_waits(nc):
    """This container's walrus accepts only ONE sem-wait per instruction
    (setupSyncWait: 'Too many sync wait commands'). Tile's wait assignment
    emits multi-wait instructions, so split: every extra wait moves onto a
    dedicated NoOp inserted just before the instruction on the same engine.
    Engine program order makes this semantically identical."""
    import concourse.mybir as mybir

    for f in nc.m.functions:
        for blk in f.blocks:
            il = blk.instructions
            i = 0
            while i < len(il):
                inst = il[i]
                si = inst.sync_info
                waits = list(si.on_wait) if si and si.on_wait else []
                if len(waits) > 1:
                    for w in waits[:-1]:
                        nop = mybir.InstNoOp(
                            name=nc.get_next_instruction_name(), ins=[], outs=[]
                        )
                        nop.engine = inst.engine
                        nop.sync_info = mybir.SyncInfo(on_wait=[w], on_update=[])
                        il.insert(i, nop)
                        i += 1
                    inst.sync_info = mybir.SyncInfo(
                        on_wait=[waits[-1]], on_update=si.on_update
                    )
                i += 1


def _build_v1(reps: int = 1, phases: str = "LTSMR"):
    """phases: cumulative subset for timing bisection — L=loads+casts,
    T=transposes, S=squares, M=matmuls, R=reduces. Default full kernel."""
    import contextlib

    import concourse.bass as bass
    import concourse.mybir as mybir
    import concourse.tile as tile
    from concourse.bass import ts

    nc = bass.Bass()
    f32 = mybir.dt.float32
    bf16 = mybir.dt.bfloat16

    qe = nc.dram_tensor("qe", [BPC, LQ, D], f32, kind="ExternalInput")
    de = nc.dram_tensor("de", [BPC, LD, D], f32, kind="ExternalInput")
    out = nc.dram_tensor("out", [1, BPC], f32, kind="ExternalOutput")

    with tile.TileContext(nc) as tc:
        with (
            tc.tile_pool(name="consts", bufs=1) as cpool,
            tc.tile_pool(name="work", bufs=3) as wpool,
            tc.tile_pool(name="acc", bufs=1) as apool,
            tc.tile_pool(name="ps", bufs=4, space="PSUM") as pspool,
        ):
            neg_half = cpool.tile([128, 128], bf16)
            nc.gpsimd.memset(neg_half, -0.5)
            ones_col = cpool.tile([128, 1], f32)
            nc.gpsimd.memset(ones_col, 1.0)

            # rowmax accumulator [128, BPC*NT] and qsq accumulator [128, BPC]
            MX = apool.tile([128, BPC * NT], f32)
            QS = apool.tile([128, BPC], f32)
            if "R" not in phases or "S" not in phases:
                nc.gpsimd.memset(MX, 0.0)
                nc.gpsimd.memset(QS, 0.0)

            GRP = 4  # batches per load/cast group
            NG = BPC // GRP
            # reps>1 is the timing mode: run the identical body in a HARDWARE
            # loop so program size (and thus NEFF load time) stays constant
            # across reps — the wall-clock differential then isolates actual
            # steady-state device execution instead of NEFF-size overhead.
            rep_ctx = tc.For_i(0, reps) if reps > 1 else contextlib.nullcontext()
            with rep_ctx:
                # Phase 1: HWDGE f32 loads, 4 batches per DMA (SWDGE
                # cast-DMAs measured ~2ms each here — pathological). Layout
                # q = NT*p + t: partition p covers NT consecutive DRAM rows =
                # one contiguous run. Tile t then holds q's {NT*p + t}, a
                # harmless reordering (scores sum over all q).
                if not phases:
                    tick = wpool.tile([128, 1], f32, tag="tick")
                    nc.gpsimd.memset(tick, 0.0)
                qbs, dbs = [], []
                for g in range(NG if "L" in phases else 0):
                    qf = wpool.tile([128, GRP, NT, 128], f32, tag="qf")
                    df = wpool.tile([128, GRP, NT, 128], f32, tag="df")
                    nc.sync.dma_start(
                        qf, qe[g * GRP : (g + 1) * GRP].rearrange("b (p t) d -> p b t d", t=NT)
                    )
                    nc.sync.dma_start(
                        df, de[g * GRP : (g + 1) * GRP].rearrange("b (p t) d -> p b t d", t=NT)
                    )
                    # Phase 2: cast f32 -> bf16 (gpsimd; 1-input ~line rate).
                    # bf16 tiles get a fresh slot per group: WAR waits would
                    # land on XPOSE DMAs, which allow a single sem-wait.
                    qb = wpool.tile([128, GRP, NT, 128], bf16, tag="qb", bufs=NG)
                    db = wpool.tile([128, GRP, NT, 128], bf16, tag="db", bufs=NG)
                    _CAST_Q(nc, qb, qf)
                    _CAST_D(nc, db, df)
                    qbs.append(qb)
                    dbs.append(db)

                # Phase 3: xbar transposes -> QT/DT [d, L] per batch
                QTs, DTs = [], []
                for b in range(BPC if "T" in phases else 0):
                    g, i = b // GRP, b % GRP
                    QT = wpool.tile([128, LQ], bf16, tag="QT", bufs=BPC)
                    DT = wpool.tile([128, LD], bf16, tag="DT", bufs=BPC)
                    for t in range(NT):
                        nc.sync.dma_start_transpose(QT[:, ts(t, 128)], qbs[g][:, i, t, :])
                        nc.sync.dma_start_transpose(DT[:, ts(t, 128)], dbs[g][:, i, t, :])
                    QTs.append(QT)
                    DTs.append(DT)

                # Phase 4: SQ = DT*DT and qsq accumulation (ScalarE)
                SQs = []
                for b in range(BPC if "S" in phases else 0):
                    SQ = wpool.tile([128, LD], bf16, tag="SQ", bufs=BPC)
                    nc.scalar.activation(SQ, DTs[b], mybir.ActivationFunctionType.Square)
                    SQs.append(SQ)
                    g, i = b // GRP, b % GRP
                    junkb = wpool.tile([128, NT, 128], bf16, tag="junkb")
                    nc.scalar.activation(
                        junkb,
                        qbs[g][:, i],
                        mybir.ActivationFunctionType.Square,
                        accum_out=QS[:, b : b + 1],
                    )

                # Phase 5: per batch, 4 accumulation groups (bias bcast + qd)
                # split across two 2-bank psum tiles (bufs=4), each closed by
                # a fused rowmax reduce. Finer PSUM granularity lets PE start
                # the next half-batch while DVE reduces the previous one —
                # measured ~19% faster than one 4-bank tile x 2 bufs.
                # (Measured alternatives on this runtime: single matmuls +
                # DVE bias-add in PSUM serialize PE<->DVE at tile granularity
                # and are ~2x slower overall than the accumulation groups.)
                for b in range(BPC if "M" in phases else 0):
                  for h in range(2):
                    pst = pspool.tile([128, NT // 2, LD], f32, tag="pst")
                    for t2 in range(NT // 2):
                        t = h * (NT // 2) + t2
                        nc.tensor.matmul(
                            pst[:, t2, :], lhsT=neg_half, rhs=SQs[b],
                            start=True, stop=False,
                        )
                        nc.tensor.matmul(
                            pst[:, t2, :],
                            lhsT=QTs[b][:, ts(t, 128)],
                            rhs=DTs[b],
                            start=False,
                            stop=True,
                        )
                    if "R" in phases:
                        nc.vector.reduce_max(
                            MX[:, b * NT + h * (NT // 2) : b * NT + (h + 1) * (NT // 2)],
                            pst, axis=mybir.AxisListType.X
                        )

            # Endgame: SC[p, b] = 2 * sum_t MX[p, b*NT+t] - QS[p, b]
            msum = apool.tile([128, BPC], f32)
            nc.vector.reduce_sum(
                msum, MX.rearrange("p (b t) -> p b t", t=NT), axis=mybir.AxisListType.X
            )
            msum2 = apool.tile([128, BPC], f32)
            nc.vector.tensor_scalar_mul(msum2, msum, 2.0)
            sc = apool.tile([128, BPC], f32)
            nc.vector.tensor_tensor(sc, msum2, QS, op=mybir.AluOpType.subtract)

            # partition sum via ones-matmul -> [1, BPC] (reuses a pst slot)
            ps_s = pspool.tile([1, BPC], f32, tag="pst")
            nc.tensor.matmul(ps_s, lhsT=ones_col, rhs=sc, start=True, stop=True)
            score = apool.tile([1, BPC], f32)
            nc.vector.tensor_copy(score, ps_s)
            nc.sync.dma_start(out[:, :], score)

    _split_multi_waits(nc)
    return nc


def _build2(
    reps: int = 1,
    n_drain: int = 8,       # batches routed ACT-drain + DVE-tree (rest: direct DVE reduce)
    qsq_eng: str = "pool",  # "pool" | "act" | "dve"
    xp_ring: str = "act",   # ring for group transposes: "sp" | "act"
    groups=(1, 1, 2, 4, 4, 4),  # batch counts per load group (sum == BPC)
):
    """v2: software-pipelined groups + group-level xbar transposes.

    Per load group g (sizes from `groups`, small first to cut pipeline ramp):
      - 1 load DMA per operand (SP ring), f32 [128, grp, NT, 128]
      - bf16 casts (Pool)
      - ONE xbar-transpose DMA per operand (out[d,(b t),q] = in[q,(b t d)];
        2 calls/group vs 8 per-batch calls: the ~1.3us fixed per-DMA cost on
        HW made 128 small transposes the dominant kernel cost)
      - per batch: SQ=DT^2 (ACT), qsq square-accum (knob engine), 2 PSUM
        accumulation groups (bias mm + 2 qd mms), then either a direct DVE
        reduce_max from PSUM or an ACT drain to bf16 SBUF + DVE k-split max
        tree (tt 2x) — splits PSUM-read work across ACT and DVE.
    Transposes issue on the ACT ring AFTER the previous group's ACT compute
    (in-order engine queue: no head-blocking stalls).
    """
    import contextlib

    import concourse.bass as bass
    import concourse.mybir as mybir
    import concourse.tile as tile

    assert sum(groups) == BPC
    nc = bass.Bass()
    f32 = mybir.dt.float32
    bf16 = mybir.dt.bfloat16

    qe = nc.dram_tensor("qe", [BPC, LQ, D], f32, kind="ExternalInput")
    de = nc.dram_tensor("de", [BPC, LD, D], f32, kind="ExternalInput")
    out = nc.dram_tensor("out", [1, BPC], f32, kind="ExternalOutput")

    NG = len(groups)
    g_off = [sum(groups[:g]) for g in range(NG)]
    # spread drained batches over the tail half, interleaved, so DVE takes
    # full batches during the pipeline ramp and sheds work once saturated
    drain_set = set()
    b = BPC - 1
    while len(drain_set) < n_drain and b >= 0:
        drain_set.add(b)
        b -= 2 if (BPC - 1 - b) < 2 * (BPC // 4) else 1
    while len(drain_set) < n_drain:
        drain_set.add(max(x for x in range(BPC) if x not in drain_set))
    xp = nc.scalar if xp_ring == "act" else nc.sync

    with tile.TileContext(nc) as tc:
        with (
            tc.tile_pool(name="consts", bufs=1) as cpool,
            tc.tile_pool(name="work", bufs=3) as wpool,
            tc.tile_pool(name="acc", bufs=1) as apool,
            tc.tile_pool(name="ps", bufs=4, space="PSUM") as pspool,
        ):
            neg_half = cpool.tile([128, 128], bf16)
            nc.gpsimd.memset(neg_half, -0.5)
            ones_col = cpool.tile([128, 1], f32)
            nc.gpsimd.memset(ones_col, 1.0)

            MX = apool.tile([128, BPC * NT], f32)
            QS = apool.tile([128, BPC], f32)

            rep_ctx = tc.For_i(0, reps) if reps > 1 else contextlib.nullcontext()
            with rep_ctx:
                qbs, dbs = {}, {}

                def do_load(g):
                    grp = groups[g]
                    o = g_off[g]
                    qf = wpool.tile([128, grp, NT, 128], f32, tag=f"qf{grp}", bufs=2)
                    df = wpool.tile([128, grp, NT, 128], f32, tag=f"df{grp}", bufs=2)
                    nc.sync.dma_start(
                        qf, qe[o : o + grp].rearrange("b (p t) d -> p b t d", t=NT)
                    )
                    nc.sync.dma_start(
                        df, de[o : o + grp].rearrange("b (p t) d -> p b t d", t=NT)
                    )
                    qb = wpool.tile([128, grp, NT, 128], bf16, tag=f"qb{grp}", bufs=2)
                    db = wpool.tile([128, grp, NT, 128], bf16, tag=f"db{grp}", bufs=2)
                    nc.gpsimd.tensor_copy(qb, qf)
                    nc.gpsimd.tensor_copy(db, df)
                    qbs[g], dbs[g] = qb, db

                def do_xp(g):
                    grp = groups[g]
                    QTg = wpool.tile([128, grp, NT, 128], bf16, tag=f"QTg{grp}", bufs=2)
                    DTg = wpool.tile([128, grp, NT, 128], bf16, tag=f"DTg{grp}", bufs=2)
                    xp.dma_start_transpose(
                        DTg.rearrange("p b t k -> p (b t) k"),
                        dbs[g].rearrange("p b t d -> p (b t d)"),
                    )
                    xp.dma_start_transpose(
                        QTg.rearrange("p b t q -> p (b t) q"),
                        qbs[g].rearrange("p b t d -> p (b t d)"),
                    )
                    return QTg, DTg

                # software pipeline, SP issue order: L0 L1 X0 L2 X1 L3 X2 ...
                # (each xp(g) issues after load(g+1) so cast(g) has a full
                # load-duration to finish -> no SP/ACT queue head stalls)
                do_load(0)
                if NG > 1:
                    do_load(1)
                xps_next = do_xp(0)
                for g, grp in enumerate(groups):
                    QTg, DTg = xps_next
                    if g + 2 < NG:
                        do_load(g + 2)
                    if g + 1 < NG:
                        xps_next = do_xp(g + 1)
                    for i in range(grp):
                        b = g_off[g] + i
                        QT = QTg[:, i]  # [128, NT, 128] = [d, t, q]
                        DT = DTg[:, i].rearrange("p t k -> p (t k)")  # [d, 512]

                        SQt = wpool.tile([128, NT, 128], bf16, tag="SQ", bufs=4)
                        nc.scalar.activation(SQt, DTg[:, i], mybir.ActivationFunctionType.Square)
                        SQ = SQt.rearrange("p t k -> p (t k)")
                        junkb = wpool.tile([128, NT, 128], bf16, tag="junkb")
                        if qsq_eng == "act":
                            nc.scalar.activation(
                                junkb, qbs[g][:, i],
                                mybir.ActivationFunctionType.Square,
                                accum_out=QS[:, b : b + 1],
                            )
                        else:
                            eng = nc.gpsimd if qsq_eng == "pool" else nc.vector
                            eng.scalar_tensor_tensor(
                                junkb, qbs[g][:, i], 1.0, qbs[g][:, i],
                                op0=mybir.AluOpType.mult, op1=mybir.AluOpType.mult,
                                accum_out=QS[:, b : b + 1],
                            )

                        drain = b in drain_set
                        for h in range(2):
                            pst = pspool.tile([128, NT // 2, LD], f32, tag="pst")
                            for t2 in range(NT // 2):
                                t = h * (NT // 2) + t2
                                nc.tensor.matmul(
                                    pst[:, t2, :], lhsT=neg_half, rhs=SQ,
                                    start=True, stop=False,
                                )
                                nc.tensor.matmul(
                                    pst[:, t2, :], lhsT=QT[:, t, :], rhs=DT,
                                    start=False, stop=True,
                                )
                            col = b * NT + h * (NT // 2)
                            if not drain:
                                nc.vector.reduce_max(
                                    MX[:, col : col + NT // 2],
                                    pst, axis=mybir.AxisListType.X,
                                )
                            else:
                                dr = wpool.tile([128, NT // 2, LD], bf16, tag="dr", bufs=4)
                                nc.scalar.copy(dr, pst)
                                X = wpool.tile([128, NT // 2, LD // 2], bf16, tag="X", bufs=4)
                                nc.vector.tensor_tensor(
                                    X, dr[:, :, 0 : LD // 2], dr[:, :, LD // 2 : LD],
                                    op=mybir.AluOpType.max,
                                )
                                Y = wpool.tile([128, NT // 2, LD // 4], bf16, tag="Y", bufs=4)
                                nc.vector.tensor_tensor(
                                    Y, X[:, :, 0 : LD // 4], X[:, :, LD // 4 : LD // 2],
                                    op=mybir.AluOpType.max,
                                )
                                nc.vector.reduce_max(
                                    MX[:, col : col + NT // 2],
                                    Y, axis=mybir.AxisListType.X,
                                )

            # Endgame: SC[p, b] = 2 * sum_t MX[p, b*NT+t] - QS[p, b]
            msum = apool.tile([128, BPC], f32)
            nc.vector.reduce_sum(
                msum, MX.rearrange("p (b t) -> p b t", t=NT), axis=mybir.AxisListType.X
            )
            msum2 = apool.tile([128, BPC], f32)
            nc.vector.tensor_scalar_mul(msum2, msum, 2.0)
            sc = apool.tile([128, BPC], f32)
            nc.vector.tensor_tensor(sc, msum2, QS, op=mybir.AluOpType.subtract)

            ps_s = pspool.tile([1, BPC], f32, tag="pst")
            nc.tensor.matmul(ps_s, lhsT=ones_col, rhs=sc, start=True, stop=True)
            score = apool.tile([1, BPC], f32)
            nc.vector.tensor_copy(score, ps_s)
            nc.sync.dma_start(out[:, :], score)

    _split_multi_waits(nc)
    return nc


BEST_CFG = dict(
    n_drain=0, qsq_eng="act", xp_ring="sp",
    groups=(1, 1, 2, 2, 2, 2, 2, 2, 2),
)


def _build(reps: int = 1):
    """Best-known configuration (see BEST_CFG). reps>1 runs the body in a
    hardware For_i loop so program size stays constant — the wall-clock
    differential in test.py then isolates steady-state device time."""
    return _build2(reps=reps, **BEST_CFG)


def kernel(query_embedding: np.ndarray, document_embedding: np.ndarray) -> np.ndarray:
    from concourse.bass_utils import run_bass_kernel_spmd

    if "nc" not in _compiled:
        _compiled["nc"] = _build()
    nc = _compiled["nc"]

    qe = np.ascontiguousarray(query_embedding, dtype=np.float32)
    de = np.ascontiguousarray(document_embedding, dtype=np.float32)
    in_maps = [
        {"qe": qe[c * BPC : (c + 1) * BPC], "de": de[c * BPC : (c + 1) * BPC]}
        for c in range(N_CORES)
    ]
    res = run_bass_kernel_spmd(nc, in_maps, core_ids=list(range(N_CORES)))
    return np.concatenate(
        [res.results[c]["out"].reshape(BPC) for c in range(N_CORES)]
    ).astype(np.float32)



# revision 20
# speedup vs baseline: 619.6943x; 1.8070x over previous
"""ColBERT late-interaction scoring kernel for Trainium2 (Bass/Tile).

score_b = sum_q max_k (2*Q@D^T - ||q||^2 - ||d||^2)
        = 2 * sum_q max_k (qd[q,k] - 0.5*d_sq[k])  -  ||Q_b||_F^2

Sharding: data-parallel over batch B=128 across 8 NeuronCores (16 each).

v2 per-core pipeline (software-pipelined load groups, see _build2):
  1. HWDGE f32 loads, one DMA per group-operand (SP ring)
  2. GPSIMD casts f32 -> bf16
  3. ONE batched xbar-transpose DMA per group-operand:
     out[d, (b t), q] = in[q, (b t d)] — transposes all the group's
     128x128 planes in a single DMA (8 transpose DMAs total vs 128
     per-tile ones; each DMA has ~1.3us fixed cost on HW)
  4. ACT squares: SQ = DT*DT (for d_sq) and ||Q||^2 accumulation
  5. PE per half-batch: accumulation group = bias matmul (lhsT=-0.5,
     rhs=SQ, broadcasts -0.5*d_sq) + 2 qd matmuls into a PSUM tile
  6. DVE reduce_max over k from PSUM (optionally: ACT drain + DVE
     2x tt-max tree for n_drain tail batches)
  Endgame: small reduces + ones-matmul partition sum -> [16] scores.

Timing note: reps>1 builds the body inside a tc.For_i HARDWARE loop so
program size (and NEFF load time) stays constant across reps — test.py's
wall-clock differential then isolates steady-state device execution.
"""

import numpy as np

B, LQ, LD, D = 128, 512, 512, 128
N_CORES = 8
BPC = B // N_CORES  # batches per core
NT = LQ // 128  # q tiles per batch

_compiled = {}


def _cast_gpsimd(nc, dst, src):
    nc.gpsimd.tensor_copy(dst, src)


def _cast_act(nc, dst, src):
    nc.scalar.copy(dst, src)


def _cast_dve(nc, dst, src):
    nc.vector.tensor_copy(dst, src)


_CAST_Q = _cast_gpsimd
_CAST_D = _cast_gpsimd


def _split_multi_waits(nc):
    """This container's walrus accepts only ONE sem-wait per instruction
    (setupSyncWait: 'Too many sync wait commands'). Tile's wait assignment
    emits multi-wait instructions, so split: every extra wait moves onto a
    dedicated NoOp inserted just before the instruction on the same engine.
    Engine program order makes this semantically identical."""
    import concourse.mybir as mybir

    for f in nc.m.functions:
        for blk in f.blocks:
            il = blk.instructions
            i = 0
            while i < len(il):
                inst = il[i]
                si = inst.sync_info
                waits = list(si.on_wait) if si and si.on_wait else []
                if len(waits) > 1:
                    for w in waits[:-1]:
                        nop = mybir.InstNoOp(
                            name=nc.get_next_instruction_name(), ins=[], outs=[]
                        )
                        nop.engine = inst.engine
                        nop.sync_info = mybir.SyncInfo(on_wait=[w], on_update=[])
                        il.insert(i, nop)
                        i += 1
                    inst.sync_info = mybir.SyncInfo(
                        on_wait=[waits[-1]], on_update=si.on_update
                    )
                i += 1


def _build_v1(reps: int = 1, phases: str = "LTSMR"):
    """phases: cumulative subset for timing bisection — L=loads+casts,
    T=transposes, S=squares, M=matmuls, R=reduces. Default full kernel."""
    import contextlib

    import concourse.bass as bass
    import concourse.mybir as mybir
    import concourse.tile as tile
    from concourse.bass import ts

    nc = bass.Bass()
    f32 = mybir.dt.float32
    bf16 = mybir.dt.bfloat16

    qe = nc.dram_tensor("qe", [BPC, LQ, D], f32, kind="ExternalInput")
    de = nc.dram_tensor("de", [BPC, LD, D], f32, kind="ExternalInput")
    out = nc.dram_tensor("out", [1, BPC], f32, kind="ExternalOutput")

    with tile.TileContext(nc) as tc:
        with (
            tc.tile_pool(name="consts", bufs=1) as cpool,
            tc.tile_pool(name="work", bufs=3) as wpool,
            tc.tile_pool(name="acc", bufs=1) as apool,
            tc.tile_pool(name="ps", bufs=4, space="PSUM") as pspool,
        ):
            neg_half = cpool.tile([128, 128], bf16)
            nc.gpsimd.memset(neg_half, -0.5)
            ones_col = cpool.tile([128, 1], f32)
            nc.gpsimd.memset(ones_col, 1.0)

            # rowmax accumulator [128, BPC*NT] and qsq accumulator [128, BPC]
            MX = apool.tile([128, BPC * NT], f32)
            QS = apool.tile([128, BPC], f32)
            if "R" not in phases or "S" not in phases:
                nc.gpsimd.memset(MX, 0.0)
                nc.gpsimd.memset(QS, 0.0)

            GRP = 4  # batches per load/cast group
            NG = BPC // GRP
            # reps>1 is the timing mode: run the identical body in a HARDWARE
            # loop so program size (and thus NEFF load time) stays constant
            # across reps — the wall-clock differential then isolates actual
            # steady-state device execution instead of NEFF-size overhead.
            rep_ctx = tc.For_i(0, reps) if reps > 1 else contextlib.nullcontext()
            with rep_ctx:
                # Phase 1: HWDGE f32 loads, 4 batches per DMA (SWDGE
                # cast-DMAs measured ~2ms each here — pathological). Layout
                # q = NT*p + t: partition p covers NT consecutive DRAM rows =
                # one contiguous run. Tile t then holds q's {NT*p + t}, a
                # harmless reordering (scores sum over all q).
                if not phases:
                    tick = wpool.tile([128, 1], f32, tag="tick")
                    nc.gpsimd.memset(tick, 0.0)
                qbs, dbs = [], []
                for g in range(NG if "L" in phases else 0):
                    qf = wpool.tile([128, GRP, NT, 128], f32, tag="qf")
                    df = wpool.tile([128, GRP, NT, 128], f32, tag="df")
                    nc.sync.dma_start(
                        qf, qe[g * GRP : (g + 1) * GRP].rearrange("b (p t) d -> p b t d", t=NT)
                    )
                    nc.sync.dma_start(
                        df, de[g * GRP : (g + 1) * GRP].rearrange("b (p t) d -> p b t d", t=NT)
                    )
                    # Phase 2: cast f32 -> bf16 (gpsimd; 1-input ~line rate).
                    # bf16 tiles get a fresh slot per group: WAR waits would
                    # land on XPOSE DMAs, which allow a single sem-wait.
                    qb = wpool.tile([128, GRP, NT, 128], bf16, tag="qb", bufs=NG)
                    db = wpool.tile([128, GRP, NT, 128], bf16, tag="db", bufs=NG)
                    _CAST_Q(nc, qb, qf)
                    _CAST_D(nc, db, df)
                    qbs.append(qb)
                    dbs.append(db)

                # Phase 3: xbar transposes -> QT/DT [d, L] per batch
                QTs, DTs = [], []
                for b in range(BPC if "T" in phases else 0):
                    g, i = b // GRP, b % GRP
                    QT = wpool.tile([128, LQ], bf16, tag="QT", bufs=BPC)
                    DT = wpool.tile([128, LD], bf16, tag="DT", bufs=BPC)
                    for t in range(NT):
                        nc.sync.dma_start_transpose(QT[:, ts(t, 128)], qbs[g][:, i, t, :])
                        nc.sync.dma_start_transpose(DT[:, ts(t, 128)], dbs[g][:, i, t, :])
                    QTs.append(QT)
                    DTs.append(DT)

                # Phase 4: SQ = DT*DT and qsq accumulation (ScalarE)
                SQs = []
                for b in range(BPC if "S" in phases else 0):
                    SQ = wpool.tile([128, LD], bf16, tag="SQ", bufs=BPC)
                    nc.scalar.activation(SQ, DTs[b], mybir.ActivationFunctionType.Square)
                    SQs.append(SQ)
                    g, i = b // GRP, b % GRP
                    junkb = wpool.tile([128, NT, 128], bf16, tag="junkb")
                    nc.scalar.activation(
                        junkb,
                        qbs[g][:, i],
                        mybir.ActivationFunctionType.Square,
                        accum_out=QS[:, b : b + 1],
                    )

                # Phase 5: per batch, 4 accumulation groups (bias bcast + qd)
                # split across two 2-bank psum tiles (bufs=4), each closed by
                # a fused rowmax reduce. Finer PSUM granularity lets PE start
                # the next half-batch while DVE reduces the previous one —
                # measured ~19% faster than one 4-bank tile x 2 bufs.
                # (Measured alternatives on this runtime: single matmuls +
                # DVE bias-add in PSUM serialize PE<->DVE at tile granularity
                # and are ~2x slower overall than the accumulation groups.)
                for b in range(BPC if "M" in phases else 0):
                  for h in range(2):
                    pst = pspool.tile([128, NT // 2, LD], f32, tag="pst")
                    for t2 in range(NT // 2):
                        t = h * (NT // 2) + t2
                        nc.tensor.matmul(
                            pst[:, t2, :], lhsT=neg_half, rhs=SQs[b],
                            start=True, stop=False,
                        )
                        nc.tensor.matmul(
                            pst[:, t2, :],
                            lhsT=QTs[b][:, ts(t, 128)],
                            rhs=DTs[b],
                            start=False,
                            stop=True,
                        )
                    if "R" in phases:
                        nc.vector.reduce_max(
                            MX[:, b * NT + h * (NT // 2) : b * NT + (h + 1) * (NT // 2)],
                            pst, axis=mybir.AxisListType.X
                        )

            # Endgame: SC[p, b] = 2 * sum_t MX[p, b*NT+t] - QS[p, b]
            msum = apool.tile([128, BPC], f32)
            nc.vector.reduce_sum(
                msum, MX.rearrange("p (b t) -> p b t", t=NT), axis=mybir.AxisListType.X
            )
            msum2 = apool.tile([128, BPC], f32)
            nc.vector.tensor_scalar_mul(msum2, msum, 2.0)
            sc = apool.tile([128, BPC], f32)
            nc.vector.tensor_tensor(sc, msum2, QS, op=mybir.AluOpType.subtract)

            # partition sum via ones-matmul -> [1, BPC] (reuses a pst slot)
            ps_s = pspool.tile([1, BPC], f32, tag="pst")
            nc.tensor.matmul(ps_s, lhsT=ones_col, rhs=sc, start=True, stop=True)
            score = apool.tile([1, BPC], f32)
            nc.vector.tensor_copy(score, ps_s)
            nc.sync.dma_start(out[:, :], score)

    _split_multi_waits(nc)
    return nc


def _build2(
    reps: int = 1,
    n_drain: int = 8,       # batches routed ACT-drain + DVE-tree (rest: direct DVE reduce)
    qsq_eng: str = "act",   # "act" | "dve" ("pool"/TensorScalarPtr fails walrus engine check on HW)
    xp_ring: str = "act",   # ring for group transposes: "sp" | "act"
    groups=(1, 1, 2, 4, 4, 4),  # batch counts per load group (sum == BPC)
):
    """v2: software-pipelined groups + group-level xbar transposes.

    Per load group g (sizes from `groups`, small first to cut pipeline ramp):
      - 1 load DMA per operand (SP ring), f32 [128, grp, NT, 128]
      - bf16 casts (Pool)
      - ONE xbar-transpose DMA per operand (out[d,(b t),q] = in[q,(b t d)];
        2 calls/group vs 8 per-batch calls: the ~1.3us fixed per-DMA cost on
        HW made 128 small transposes the dominant kernel cost)
      - per batch: SQ=DT^2 (ACT), qsq square-accum (knob engine), 2 PSUM
        accumulation groups (bias mm + 2 qd mms), then either a direct DVE
        reduce_max from PSUM or an ACT drain to bf16 SBUF + DVE k-split max
        tree (tt 2x) — splits PSUM-read work across ACT and DVE.
    Transposes issue on the ACT ring AFTER the previous group's ACT compute
    (in-order engine queue: no head-blocking stalls).
    """
    import contextlib

    import concourse.bass as bass
    import concourse.mybir as mybir
    import concourse.tile as tile

    assert sum(groups) == BPC
    nc = bass.Bass()
    f32 = mybir.dt.float32
    bf16 = mybir.dt.bfloat16

    qe = nc.dram_tensor("qe", [BPC, LQ, D], f32, kind="ExternalInput")
    de = nc.dram_tensor("de", [BPC, LD, D], f32, kind="ExternalInput")
    out = nc.dram_tensor("out", [1, BPC], f32, kind="ExternalOutput")

    NG = len(groups)
    g_off = [sum(groups[:g]) for g in range(NG)]
    # single-buffer the big per-group tiles when groups are huge (SBUF cap)
    gbufs = 2 if max(groups) <= 8 else 1
    # spread drained batches over the tail half, interleaved, so DVE takes
    # full batches during the pipeline ramp and sheds work once saturated
    drain_set = set()
    b = BPC - 1
    while len(drain_set) < n_drain and b >= 0:
        drain_set.add(b)
        b -= 2 if (BPC - 1 - b) < 2 * (BPC // 4) else 1
    while len(drain_set) < n_drain:
        drain_set.add(max(x for x in range(BPC) if x not in drain_set))
    xp = nc.scalar if xp_ring == "act" else nc.sync

    with tile.TileContext(nc) as tc:
        with (
            tc.tile_pool(name="consts", bufs=1) as cpool,
            tc.tile_pool(name="work", bufs=3) as wpool,
            tc.tile_pool(name="acc", bufs=1) as apool,
            tc.tile_pool(name="ps", bufs=4, space="PSUM") as pspool,
        ):
            neg_half = cpool.tile([128, 128], bf16)
            nc.gpsimd.memset(neg_half, -0.5)
            ones_col = cpool.tile([128, 1], f32)
            nc.gpsimd.memset(ones_col, 1.0)

            MX = apool.tile([128, BPC * NT], f32)
            QS = apool.tile([128, BPC], f32)

            rep_ctx = tc.For_i(0, reps) if reps > 1 else contextlib.nullcontext()
            with rep_ctx:
                qbs, dbs = {}, {}

                def do_load(g):
                    grp = groups[g]
                    o = g_off[g]
                    qf = wpool.tile([128, grp, NT, 128], f32, tag=f"qf{grp}", bufs=gbufs)
                    df = wpool.tile([128, grp, NT, 128], f32, tag=f"df{grp}", bufs=gbufs)
                    nc.sync.dma_start(
                        qf, qe[o : o + grp].rearrange("b (p t) d -> p b t d", t=NT)
                    )
                    nc.sync.dma_start(
                        df, de[o : o + grp].rearrange("b (p t) d -> p b t d", t=NT)
                    )
                    qb = wpool.tile([128, grp, NT, 128], bf16, tag=f"qb{grp}", bufs=gbufs)
                    db = wpool.tile([128, grp, NT, 128], bf16, tag=f"db{grp}", bufs=gbufs)
                    nc.gpsimd.tensor_copy(qb, qf)
                    nc.gpsimd.tensor_copy(db, df)
                    qbs[g], dbs[g] = qb, db

                def do_xp(g):
                    grp = groups[g]
                    QTg = wpool.tile([128, grp, NT, 128], bf16, tag=f"QTg{grp}", bufs=gbufs)
                    DTg = wpool.tile([128, grp, NT, 128], bf16, tag=f"DTg{grp}", bufs=gbufs)
                    xp.dma_start_transpose(
                        DTg.rearrange("p b t k -> p (b t) k"),
                        dbs[g].rearrange("p b t d -> p (b t d)"),
                    )
                    xp.dma_start_transpose(
                        QTg.rearrange("p b t q -> p (b t) q"),
                        qbs[g].rearrange("p b t d -> p (b t d)"),
                    )
                    return QTg, DTg

                # software pipeline, SP issue order: L0 L1 X0 L2 X1 L3 X2 ...
                # (each xp(g) issues after load(g+1) so cast(g) has a full
                # load-duration to finish -> no SP/ACT queue head stalls)
                do_load(0)
                if NG > 1:
                    do_load(1)
                xps_next = do_xp(0)
                for g, grp in enumerate(groups):
                    QTg, DTg = xps_next
                    if g + 2 < NG:
                        do_load(g + 2)
                    if g + 1 < NG:
                        xps_next = do_xp(g + 1)
                    for i in range(grp):
                        b = g_off[g] + i
                        QT = QTg[:, i]  # [128, NT, 128] = [d, t, q]
                        DT = DTg[:, i].rearrange("p t k -> p (t k)")  # [d, 512]

                        SQt = wpool.tile([128, NT, 128], bf16, tag="SQ", bufs=4)
                        nc.scalar.activation(SQt, DTg[:, i], mybir.ActivationFunctionType.Square)
                        SQ = SQt.rearrange("p t k -> p (t k)")
                        junkb = wpool.tile([128, NT, 128], bf16, tag="junkb")
                        if qsq_eng == "act":
                            nc.scalar.activation(
                                junkb, qbs[g][:, i],
                                mybir.ActivationFunctionType.Square,
                                accum_out=QS[:, b : b + 1],
                            )
                        else:
                            eng = nc.gpsimd if qsq_eng == "pool" else nc.vector
                            eng.scalar_tensor_tensor(
                                junkb, qbs[g][:, i], 1.0, qbs[g][:, i],
                                op0=mybir.AluOpType.mult, op1=mybir.AluOpType.mult,
                                accum_out=QS[:, b : b + 1],
                            )

                        drain = b in drain_set
                        for h in range(2):
                            pst = pspool.tile([128, NT // 2, LD], f32, tag="pst")
                            for t2 in range(NT // 2):
                                t = h * (NT // 2) + t2
                                nc.tensor.matmul(
                                    pst[:, t2, :], lhsT=neg_half, rhs=SQ,
                                    start=True, stop=False,
                                )
                                nc.tensor.matmul(
                                    pst[:, t2, :], lhsT=QT[:, t, :], rhs=DT,
                                    start=False, stop=True,
                                )
                            col = b * NT + h * (NT // 2)
                            if not drain:
                                nc.vector.reduce_max(
                                    MX[:, col : col + NT // 2],
                                    pst, axis=mybir.AxisListType.X,
                                )
                            else:
                                dr = wpool.tile([128, NT // 2, LD], bf16, tag="dr", bufs=4)
                                nc.scalar.copy(dr, pst)
                                X = wpool.tile([128, NT // 2, LD // 2], bf16, tag="X", bufs=4)
                                nc.vector.tensor_tensor(
                                    X, dr[:, :, 0 : LD // 2], dr[:, :, LD // 2 : LD],
                                    op=mybir.AluOpType.max,
                                )
                                Y = wpool.tile([128, NT // 2, LD // 4], bf16, tag="Y", bufs=4)
                                nc.vector.tensor_tensor(
                                    Y, X[:, :, 0 : LD // 4], X[:, :, LD // 4 : LD // 2],
                                    op=mybir.AluOpType.max,
                                )
                                nc.vector.reduce_max(
                                    MX[:, col : col + NT // 2],
                                    Y, axis=mybir.AxisListType.X,
                                )

            # Endgame: SC[p, b] = 2 * sum_t MX[p, b*NT+t] - QS[p, b]
            msum = apool.tile([128, BPC], f32)
            nc.vector.reduce_sum(
                msum, MX.rearrange("p (b t) -> p b t", t=NT), axis=mybir.AxisListType.X
            )
            msum2 = apool.tile([128, BPC], f32)
            nc.vector.tensor_scalar_mul(msum2, msum, 2.0)
            sc = apool.tile([128, BPC], f32)
            nc.vector.tensor_tensor(sc, msum2, QS, op=mybir.AluOpType.subtract)

            ps_s = pspool.tile([1, BPC], f32, tag="pst")
            nc.tensor.matmul(ps_s, lhsT=ones_col, rhs=sc, start=True, stop=True)
            score = apool.tile([1, BPC], f32)
            nc.vector.tensor_copy(score, ps_s)
            nc.sync.dma_start(out[:, :], score)

    _split_multi_waits(nc)
    return nc


# HW A/B (rep-slope at reps=2001, min-of-5): groups (4,4,4,4) -> 77us/rep,
# (1,1,2,2,2,2,2,2,2) -> 121us, (8,8) -> 132us, (16,) -> 298us (bufs=1
# serializes reps), n_drain=6 -> 181us. HW is DMA-fixed-cost dominated:
# 4 groups x (2 loads + 2 batched transposes) is the sweet spot.
BEST_CFG = dict(
    n_drain=0, qsq_eng="act", xp_ring="sp",
    groups=(4, 4, 4, 4),
)


def _build(reps: int = 1):
    """Best-known configuration (see BEST_CFG). reps>1 runs the body in a
    hardware For_i loop so program size stays constant — the wall-clock
    differential in test.py then isolates steady-state device time."""
    return _build2(reps=reps, **BEST_CFG)


def kernel(query_embedding: np.ndarray, document_embedding: np.ndarray) -> np.ndarray:
    from concourse.bass_utils import run_bass_kernel_spmd

    if "nc" not in _compiled:
        _compiled["nc"] = _build()
    nc = _compiled["nc"]

    qe = np.ascontiguousarray(query_embedding, dtype=np.float32)
    de = np.ascontiguousarray(document_embedding, dtype=np.float32)
    in_maps = [
        {"qe": qe[c * BPC : (c + 1) * BPC], "de": de[c * BPC : (c + 1) * BPC]}
        for c in range(N_CORES)
    ]
    res = run_bass_kernel_spmd(nc, in_maps, core_ids=list(range(N_CORES)))
    return np.concatenate(
        [res.results[c]["out"].reshape(BPC) for c in range(N_CORES)]
    ).astype(np.float32)

